# revision 29
# baseline (speedup 1.0000x reference)
"""CapsLayer2D dynamic-routing kernel for 8x TRN2 NeuronCores.

Problem (hardcoded shapes):
  inputs: [B=16, R=8, C=8, I=128, DIN=16] fp32
  W:      [K=32, I=128, DIN=16, DOUT=16] fp32
  out:    [B, R, C, K, DOUT] fp32

Math (reference does 3-round dynamic routing). Closed form (verified vs
reference to ~6e-6 rel):
  U[p,k]    = res[p,k,:,:]  (I x O per position p=(b,r,c) and k)
  s0        = mean_i U_i
  v0        = squash(s0)
  t_a = U v0        ; m_a = U^T t_a ; s1 = s0 + m_a
  v1 = squash(s1)   ; vs = v0 + v1
  t_b = U vs        ; m_b = U^T t_b ; s2 = s0 + m_b
  out = squash(s2)

Sharding: batch across 8 cores (2 batches = 128 positions per core), W
replicated. No collectives.

Per-core plan (v2):
  Host pre-transposes/casts (fp16):
    xt  [(i,d32) rows (chunk c of 128), p]      2 chunks-major layout
    w_r [(i,d32) rows (chunk c of 128), (k,o)]  32 chunks
  Device: s0 via 32 full-depth matmuls; res [p,(k,i,o)] fp16 via 128
  per-i matmuls (copies alternate Scalar/Vector engines). Routing per
  k-group of 8 on DVE with tree-adds (2x mode) instead of
  tensor_reduce (1x) and contiguous-layout muls (the baseline's
  transposed-view muls ran at ~4.3 cyc/elem).
"""

import sys

import numpy as np

sys.path.insert(0, "/opt/trn_rl_repo")

import ml_dtypes  # noqa: E402

P, I, D, K, O = 128, 128, 16, 32, 16
D2 = 32  # padded d
ID2, KO = I * D2, K * O  # 4096, 512
NCH = ID2 // 128  # 32 chunks of 128 (i,d32) rows
KC = 16  # k-group size
NG = K // KC  # 4 groups
GW = KC * O  # 128 group output width
KI = KC * I  # 1024
N_CORES = 8
EPS = 1e-7

# ut-pass mul variant: "direct" = innermost step-0 broadcast operand,
# "expand" = copy-expand t over o first, then 2x mul
VARIANT_UT = "direct"

_PROGRAM = None


def _build_program():
    from contextlib import ExitStack

    import concourse.bass as bass
    import concourse.tile as tile
    from concourse import bacc, mybir

    F32 = mybir.dt.float32
    BF16 = mybir.dt.float16  # fp16: same speed class as bf16, finer mantissa
    MULT = mybir.AluOpType.mult
    ADD = mybir.AluOpType.add
    X = mybir.AxisListType.X
    SQRT = mybir.ActivationFunctionType.Sqrt

    nc = bacc.Bacc("TRN2", target_bir_lowering=False, debug=False)

    xt_d = nc.dram_tensor("xt", [128, NCH * 128], BF16, kind="ExternalInput").ap()
    wr_d = nc.dram_tensor("wr", [128, NCH * KO], BF16, kind="ExternalInput").ap()
    out_d = nc.dram_tensor("out", [P, KO], F32, kind="ExternalOutput").ap()

    with ExitStack() as ctx:
        tc = ctx.enter_context(tile.TileContext(nc))

        pp_s = ctx.enter_context(tc.tile_pool(name="pp_s", bufs=1, space="PSUM"))
        pp_r = ctx.enter_context(tc.tile_pool(name="pp_r", bufs=3, space="PSUM"))

        rp = ctx.enter_context(tc.tile_pool(name="resp", bufs=1))
        sm0 = ctx.enter_context(tc.tile_pool(name="sm0", bufs=1))
        lhs_ctx = ExitStack()
        lhs = lhs_ctx.enter_context(tc.tile_pool(name="lhs", bufs=1))

        # ---- load pre-transposed operands (host-prepped fp16) ----
        # one xt DMA + four wr DMAs: low Sync-queue issue cost, while the
        # 4-way wr split still lets early matmuls start before the tail
        # chunks land
        xt_t = lhs.tile([128, NCH * 128], BF16, tag="xt", name="xt")
        wr_t = [lhs.tile([128, 4 * 1024], BF16, tag=f"wr{q}", name=f"wr{q}")
                for q in range(4)]
        zz = lhs.tile([128, KO], BF16, tag="zz", name="zz")
        nc.vector.memset(zz[:], 0.0)
        nc.sync.dma_start(xt_t[:], xt_d)
        for q in range(4):
            nc.sync.dma_start(wr_t[q][:], wr_d[:, q * 4096:(q + 1) * 4096])

        def xt_sl(c, r0=0, rn=128):
            return xt_t[r0:r0 + rn, c * 128:(c + 1) * 128]

        def wr_sl(c, r0=0, rn=128):
            return wr_t[c // 8][r0:r0 + rn, (c % 8) * KO:(c % 8 + 1) * KO]

        # ---- s0 (all k) ----
        ps0 = pp_s.tile([P, KO], F32, tag="ps0")
        # PE warm-up on zeros while the DMAs land: ~9us of sustained matmul
        # releases the HAM clock throttle (1.2 -> 2.4 GHz) before the real
        # stream starts; ps0 is cleared by the first start=True below.
        for _ in range(20):
            nc.tensor.matmul(ps0[:], zz[:, 0:128], zz[:], start=True, stop=True)
        eps_t = sm0.tile([P, 1], F32, tag="eps")
        nc.vector.memset(eps_t[:], EPS)

        def squash(sm, s_ap, v_ap, kw, tag):
            """v = squash(s) over o; v_ap doubles as the |s|^2 scratch."""
            nc.vector.tensor_mul(v_ap, s_ap, s_ap)
            sq = sm.tile([P, kw], F32, tag=f"sq_{tag}")
            nc.vector.tensor_reduce(
                sq[:], v_ap.rearrange("p (k o) -> p k o", k=kw), X, ADD
            )
            a = sm.tile([P, kw], F32, tag=f"sqa_{tag}")
            nc.scalar.activation(a[:], sq[:], SQRT, bias=eps_t[:])
            b = sm.tile([P, kw], F32, tag=f"sqb_{tag}")
            nc.vector.scalar_tensor_tensor(b[:], sq[:], 1.0, a[:], ADD, MULT)
            r = sm.tile([P, kw], F32, tag=f"sqr_{tag}")
            nc.vector.reciprocal(r[:], b[:])
            f = sm.tile([P, kw], F32, tag=f"sqf_{tag}")
            nc.vector.tensor_mul(f[:], sq[:], r[:])
            nc.vector.tensor_mul(
                v_ap.rearrange("p (k o) -> p k o", k=kw),
                s_ap.rearrange("p (k o) -> p k o", k=kw),
                f[:].unsqueeze(2).broadcast_to([P, kw, O]),
            )

        # ---- res production: per-i matmuls, copies on Scalar+Vector ----
        res = rp.tile([P, K * I * O], BF16)
        resv = res[:].rearrange("p (k i o) -> p k i o", k=K, i=I, o=O)
        for i0 in range(0, I, 2):
            pr = pp_r.tile([P, 2 * KO], F32, tag="pr")
            for j in range(2):
                i = i0 + j
                c, r0 = i // 4, (i % 4) * 32
                nc.tensor.matmul(
                    pr[:, j * KO:(j + 1) * KO],
                    xt_sl(c, r0, 32),
                    wr_sl(c, r0, 32),
                    start=True,
                    stop=True,
                    tile_position=(r0, 0),
                )
            src_ap = (
                pr[:]
                .rearrange("p (i2 k o) -> p i2 k o", i2=2, k=K)
                .transpose([0, 2, 1, 3])
            )
            if i0 % 4:
                nc.scalar.copy(resv[:, :, i0:i0 + 2, :], src_ap)
            else:
                nc.vector.tensor_copy(resv[:, :, i0:i0 + 2, :], src_ap)

        # ---- s0 (all k): after res production so res matmuls/copies
        # start as soon as the first weight DMA lands; s0 + squash overlap
        # the copy drain ----
        for c in range(NCH):
            nc.tensor.matmul(
                ps0[:],
                xt_sl(c),
                wr_sl(c),
                start=(c == 0),
                stop=(c == NCH - 1),
            )
        s0_all = sm0.tile([P, KO], F32)
        nc.scalar.mul(s0_all[:], ps0[:], 1.0 / I)
        v0b_all = sm0.tile([P, KO], BF16)
        with nc.allow_low_precision(reason="fp16 routing intermediates"):
            squash(sm0, s0_all[:], v0b_all[:], K, "v0")

        # ---- routing (Xt/W_r freed) ----
        # All-DVE routing (GpSimd tails measured 3-6x slower than DVE on
        # these APs and SBUF-port contention slowed concurrent DVE ops).
        # Wide groups (KC=16, two groups) halve per-op init/semaphore
        # overhead; total streamed elements are unchanged.
        lhs_ctx.close()
        sm = ctx.enter_context(tc.tile_pool(name="small", bufs=1))

        def uv_pass(g, vb_ap):
            """paired t[p,(k,i),2] = sum_o rv*vb via 2x mul + o-tree."""
            rv = resv[:, g * KC:(g + 1) * KC]
            tmp = sm.tile([P, KC * I * O], BF16, tag="tmp", name="tmp")
            tmp4 = tmp[:].rearrange("p (k i o) -> p k i o", k=KC, i=I)
            nc.vector.tensor_mul(
                tmp4, rv, vb_ap.unsqueeze(2).broadcast_to([P, KC, I, O])
            )
            nc.vector.tensor_add(tmp4[:, :, :, 0:8], tmp4[:, :, :, 0:8],
                                 tmp4[:, :, :, 8:16])
            nc.vector.tensor_add(tmp4[:, :, :, 0:4], tmp4[:, :, :, 0:4],
                                 tmp4[:, :, :, 4:8])
            nc.vector.tensor_add(tmp4[:, :, :, 0:2], tmp4[:, :, :, 0:2],
                                 tmp4[:, :, :, 2:4])
            pv = tmp[:].rearrange("p (n o) -> p n o", o=O)
            t_t = sm.tile([P, 2 * KI], BF16, tag="t", name="t")
            tp = t_t[:].rearrange("p (n two) -> p n two", two=2)
            nc.vector.tensor_add(
                tp,
                pv[:, :, 0].unsqueeze(2).broadcast_to([P, KI, 2]),
                pv[:, :, 1].unsqueeze(2).broadcast_to([P, KI, 2]),
            )
            return t_t

        def ut_pass(g, t_t):
            """m[p,(k,o)] = sum_i rv*t via 2x mul (paired-t) + i-tree."""
            rv = resv[:, g * KC:(g + 1) * KC]
            tmp = sm.tile([P, KC * I * O], BF16, tag="tmp", name="tmp")
            tmp4 = tmp[:].rearrange("p (k i o) -> p k i o", k=KC, i=I)
            tmp_pair = tmp[:].rearrange("p (n a b) -> p n a b", a=O // 2, b=2)
            rv_pair = rv.rearrange("p k i (a b) -> p (k i) a b", b=2)
            tb = (
                t_t[:]
                .rearrange("p (n two) -> p n two", two=2)
                .unsqueeze(2)
                .broadcast_to([P, KI, O // 2, 2])
            )
            nc.vector.tensor_mul(tmp_pair, rv_pair, tb)
            h = I // 2
            while h >= 1:
                nc.vector.tensor_add(
                    tmp4[:, :, 0:h, :], tmp4[:, :, 0:h, :], tmp4[:, :, h:2 * h, :]
                )
                h //= 2
            m_t = sm.tile([P, GW], BF16, tag="m", name="m")
            nc.vector.tensor_copy(
                m_t[:].rearrange("p (k o) -> p k o", k=KC), tmp4[:, :, 0, :]
            )
            return m_t

        def dve_mid(g, m_a):
            """s1 = s0+m_a; v1 = squash(s1); vs = v0+v1 (in place); vsb."""
            s0 = s0_all[:, g * GW:(g + 1) * GW]
            s1 = sm.tile([P, GW], F32, tag="s", name="s")
            nc.vector.tensor_add(s1[:], s0, m_a[:])
            v1 = sm.tile([P, GW], F32, tag="v1", name="v1")
            squash(sm, s1[:], v1[:], KC, "mid")
            nc.vector.tensor_add(
                v1[:], v1[:], v0b_all[:, g * GW:(g + 1) * GW]
            )
            vsb = sm.tile([P, GW], BF16, tag="vsb", name="vsb")
            nc.vector.tensor_copy(vsb[:], v1[:])
            return vsb[:].rearrange("p (k o) -> p k o", k=KC)

        def dve_out(g, m_b):
            s0 = s0_all[:, g * GW:(g + 1) * GW]
            s2 = sm.tile([P, GW], F32, tag="s", name="s")
            nc.vector.tensor_add(s2[:], s0, m_b[:])
            outt = sm.tile([P, GW], F32, tag="v1", name="outt")
            squash(sm, s2[:], outt[:], KC, "fin")
            nc.sync.dma_start(out_d[:, g * GW:(g + 1) * GW], outt[:])

        with nc.allow_low_precision(reason="fp16 routing intermediates"):
            for g in range(NG):
                v0b_g = v0b_all[:, g * GW:(g + 1) * GW].rearrange(
                    "p (k o) -> p k o", k=KC
                )
                t_a = uv_pass(g, v0b_g)
                m_a = ut_pass(g, t_a)
                vsb_g = dve_mid(g, m_a)
                t_b = uv_pass(g, vsb_g)
                m_b = ut_pass(g, t_b)
                dve_out(g, m_b)

    nc.compile()
    return nc


def _get_program():
    global _PROGRAM
    if _PROGRAM is None:
        _PROGRAM = _build_program()
    return _PROGRAM


def _prep_inputs(x, W):
    """Host-side layout/dtype prep: build per-core transposed activations
    and the replicated transposed weight, both fp16, d padded 16->32."""
    # xt: [(i,d32) chunk-major rows, p] per core.
    # chunk c covers i in [4c, 4c+4); row within chunk = (i%4)*32 + d.
    xs = x.reshape(N_CORES, P, I, D)  # [core, p, i, d]
    xt = np.zeros((N_CORES, NCH, 4, D2, P), dtype=np.float16)
    src = xs.transpose(0, 2, 3, 1).reshape(N_CORES, NCH, 4, D, P)
    xt[:, :, :, 0:D, :] = src.astype(np.float16)
    xt = np.ascontiguousarray(
        xt.reshape(N_CORES, NCH, 128, P).transpose(0, 2, 1, 3).reshape(
            N_CORES, 128, NCH * P
        )
    )
    # wr: [(i,d32) rows, (k,o)] per chunk
    wr = np.zeros((NCH, 4, D2, K, O), dtype=np.float16)
    wsrc = W.transpose(1, 2, 0, 3).reshape(NCH, 4, D, K, O)  # [c,i4,d,k,o]
    wr[:, :, 0:D] = wsrc.astype(np.float16)
    wr = np.ascontiguousarray(
        wr.reshape(NCH, 128, KO).transpose(1, 0, 2).reshape(128, NCH * KO)
    )
    return xt, wr


def kernel(**inputs):
    x = np.ascontiguousarray(np.asarray(inputs["inputs"], dtype=np.float32))
    W = np.ascontiguousarray(np.asarray(inputs["W"], dtype=np.float32))
    assert x.shape == (16, 8, 8, 128, 16) and W.shape == (32, 128, 16, 16)

    from concourse.bass_utils import run_bass_kernel_spmd

    nc = _get_program()
    xt, wr = _prep_inputs(x, W)
    in_maps = [
        {"xt": np.ascontiguousarray(xt[c]), "wr": wr} for c in range(N_CORES)
    ]
    r = run_bass_kernel_spmd(nc, in_maps, list(range(N_CORES)))
    outs = [r.results[c]["out"].reshape(2, 8, 8, K, O) for c in range(N_CORES)]
    return np.concatenate(outs, axis=0).astype(np.float32)
